# revision 14
# baseline (speedup 1.0000x reference)
"""Trainium2 kernel for DSN (deep subspace networks) few-shot classification.

Math: reference computes, per class w with orthonormal subspace basis U_w
([d, k]), dist_w(q) = ||q - U_w U_w^T q||^2 = ||q||^2 - ||U_w^T q||^2 and
returns log_softmax(-dist) over classes. The -||q||^2 term is constant per
row, so log_softmax(-dist)[q, :] == log_softmax(s)[q, :] with
s[q, w] = ||U_w^T q||^2.

Host (tiny): group support by class, SVD -> U_w, stack W = [U_0 .. U_4]
([1024, 45]), cast to fp16, pre-transpose the query matrix.
Device (memory-bound): per core, C^T = W^T Q^T ([45, q]) via PE matmuls,
square, group-sum via indicator matmul ([q, 5]), rowwise log_softmax.

Sharding: data-parallel over the 16384 query rows, 2048 per core, SPMD on
8 NeuronCores. No cross-core communication.
"""

import numpy as np

import concourse.bass as bass
import concourse.bacc as bacc
import concourse.mybir as mybir
from concourse.tile import TileContext
from concourse.vector_clock import ScopedClock
from concourse.bass_utils import run_bass_kernel_spmd


class FastTileContext(TileContext):
    """TileContext with a slim kernel tail.

    The stock tail is drain -> all-engine barrier -> semaphore clear ->
    all-engine barrier (~10 us of EVSEM butterflies). The Bass preamble
    already clears the whole bass semaphore range at kernel start, so for
    a one-shot kernel the trailing clear + barriers are redundant; the
    drain (which waits on the global vector clock, i.e. every engine and
    DMA queue) is what guarantees completion.
    """

    def _drain_and_barrier(self, tick_clock, wait_clock):
        drain_inst = self.nc.sync.drain()
        wait_clock.add_sem_waits(
            drain_inst.ins, ScopedClock({None: tick_clock.global_clock})
        )
        popped = self.nc._tile_sem_poison_stack.pop()
        assert popped is self._sem_poison

# Problem geometry (hardcoded per spec).
N_CORES = 8
N_QUERY = 16384
D = 1024
N_WAY = 5
N_SHOT = 10
K = N_SHOT - 1            # 9 basis vectors per class
M = N_WAY * K             # 45 stacked basis columns
NQ = N_QUERY // N_CORES   # 2048 query rows per core
DC = D // 128             # 8 contraction chunks of 128
NB = NQ // 512            # 4 query blocks of 512 per core
SUB = 512 // 128          # 4 sub-blocks of 128 rows per block
CG = 4                    # d-chunks per input DMA piece (2 DMAs per block)

FP16 = mybir.dt.float16
FP32 = mybir.dt.float32
AX = mybir.AxisListType
AF = mybir.ActivationFunctionType
ALU = mybir.AluOpType

_CACHE = {}


def _build_bass():
    nc = bacc.Bacc("TRN2", target_bir_lowering=False, debug=False,
                   num_devices=N_CORES)
    qt = nc.declare_dram_parameter("qt", [D, NQ], FP16, isOutput=False)
    wfull = nc.declare_dram_parameter("wfull", [128, DC * M + N_WAY], FP16,
                                      isOutput=False)
    out = nc.declare_dram_parameter("out", [NQ, N_WAY], FP32, isOutput=True)

    with FastTileContext(nc) as tc:
        with (
            tc.tile_pool(name="const", bufs=1) as cpool,
            tc.tile_pool(name="qp", bufs=1) as qpool,
            tc.tile_pool(name="wk", bufs=4) as wk,
            tc.tile_pool(name="ps_ct", bufs=2, space="PSUM") as ps_ct,
            tc.tile_pool(name="ps_s", bufs=2, space="PSUM") as ps_s,
        ):
            wtile = cpool.tile([128, DC * M + N_WAY], FP16)
            nc.sync.dma_start(out=wtile, in_=wfull[:, :])
            ind = wtile[0:M, DC * M:DC * M + N_WAY]      # [45, 5]

            qtile = qpool.tile([128, DC, NQ], FP16)      # 4 MB resident
            out_acc = qpool.tile([128, NB, SUB, N_WAY], FP32)
            ssum_all = qpool.tile([128, NB * SUB], FP32)
            dma_engines = [nc.sync, nc.scalar]
            di = 0
            for b in range(NB):
                for g in range(DC // CG):
                    src = qt[g * CG * 128:(g + 1) * CG * 128,
                             b * 512:(b + 1) * 512]
                    dma_engines[di % len(dma_engines)].dma_start(
                        out=qtile[:, g * CG:(g + 1) * CG, b * 512:(b + 1) * 512],
                        in_=src.rearrange("(c p) q -> p c q", p=128),
                    )
                    di += 1

            for b in range(NB):
                qs = slice(b * 512, (b + 1) * 512)
                ct = ps_ct.tile([M, 512], FP32, tag="ct")
                for c in range(DC):
                    nc.tensor.matmul(
                        ct,
                        lhsT=wtile[:, c * M:(c + 1) * M],
                        rhs=qtile[:, c, qs],
                        start=(c == 0),
                        stop=(c == DC - 1),
                    )
                ctsq = wk.tile([M, 512], FP16, tag="ctsq")
                nc.scalar.activation(ctsq, ct, AF.Square)

                sps = ps_s.tile([128, SUB, N_WAY], FP32, tag="sps")
                for s in range(SUB):
                    nc.tensor.matmul(
                        sps[:, s, :],
                        lhsT=ctsq[:, s * 128:(s + 1) * 128],
                        rhs=ind,
                        start=True,
                        stop=True,
                    )

                negm = wk.tile([128, SUB], FP32, tag="negm")
                nc.vector.reduce_max(negm, sps, axis=AX.X, negate=True)
                nc.vector.tensor_tensor(
                    out_acc[:, b], sps,
                    negm.unsqueeze(2).broadcast_to((128, SUB, N_WAY)),
                    op=ALU.add,
                )
                ex = wk.tile([128, SUB, N_WAY], FP32, tag="ex")
                nc.scalar.activation(ex, out_acc[:, b], AF.Exp)
                nc.vector.reduce_sum(ssum_all[:, b * SUB:(b + 1) * SUB], ex,
                                     axis=AX.X)

            # One deferred Ln for all 16 row-groups (a single ACT-table
            # switch instead of one per Exp/Ln interleave), then one
            # broadcast subtract over the whole output block.
            lse_all = qpool.tile([128, NB * SUB], FP32)
            nc.scalar.activation(lse_all, ssum_all, AF.Ln)
            oview = out_acc.rearrange("p b s w -> p (b s) w")
            nc.vector.tensor_tensor(
                oview, oview,
                lse_all.unsqueeze(2).broadcast_to((128, NB * SUB, N_WAY)),
                op=ALU.subtract,
            )
            nc.sync.dma_start(
                out=out[:, :].rearrange("(b s p) w -> p b s w", b=NB, s=SUB,
                                        p=128),
                in_=out_acc,
            )
    nc.compile()
    return nc


def _host_prep(train_imgs, train_labels, query_imgs):
    """Per-class subspace bases (tiny SVDs) + fp16 device operands."""
    n_support, n_way = train_labels.shape
    n_shot = n_support // n_way
    cls = np.argmax(np.asarray(train_labels), axis=1)
    order = np.argsort(cls, kind="stable")
    grouped = np.asarray(train_imgs, np.float64)[order].reshape(
        n_way, n_shot, -1)
    mats = np.swapaxes(grouped, 1, 2)                    # [w, d, s]
    U, _, _ = np.linalg.svd(mats, full_matrices=False)   # [w, d, s]
    W = np.concatenate([U[w][:, :n_shot - 1] for w in range(n_way)],
                       axis=1)                           # [d, 45]

    # Device layout: wfull[p, c*45 + m] = W[c*128 + p, m]; indicator appended.
    wfull = np.zeros((128, DC * M + N_WAY), np.float16)
    wfull[:, :DC * M] = (
        W.reshape(DC, 128, M).transpose(1, 0, 2).reshape(128, DC * M)
    ).astype(np.float16)
    for w in range(N_WAY):
        wfull[w * K:(w + 1) * K, DC * M + w] = 1.0

    qh = np.asarray(query_imgs, np.float32).astype(np.float16)
    return wfull, qh


def _run(inputs, trace=False, **kwargs):
    if "nc" not in _CACHE:
        _CACHE["nc"] = _build_bass()
    nc = _CACHE["nc"]

    wfull, qh = _host_prep(inputs["train_imgs"], inputs["train_labels"],
                           inputs["query_imgs"])
    in_maps = []
    for k in range(N_CORES):
        shard = np.ascontiguousarray(qh[k * NQ:(k + 1) * NQ].T)  # [D, NQ]
        in_maps.append({"qt": shard, "wfull": wfull})

    res = run_bass_kernel_spmd(nc, in_maps, core_ids=list(range(N_CORES)),
                               trace=trace, **kwargs)
    full = np.concatenate([res.results[k]["out"] for k in range(N_CORES)],
                          axis=0)
    return full, res


def kernel(**inputs) -> np.ndarray:
    out, _ = _run(inputs)
    return out
